# revision 33
# baseline (speedup 1.0000x reference)
"""Causal multi-head attention (16 heads, hd=64) on 8 trn2 NeuronCores.

Sharding: core c -> batch b = c // 4, head-group g = c % 4 (4 heads = 256
columns of Wq/Wk/Wv).  Each core computes its [S, 256] slice of the three
outputs (attn out, K_cache, V_cache); the host gathers slices.

Per-core pipeline (Tile framework), all matmuls bf16 x bf16 -> f32 psum.
The trn2 PE dual-issues adjacent independent matmuls (~2 rows/cycle
aggregate), so every emission pairs independent matmuls: K/Q projection
chunks alternate between two psum tiles, V waves run two k-tiles
abreast, score/AV matmuls alternate heads.

  - x and weights arrive host-packed in SBUF layout ([p, j, ...], bf16)
    so the load DMAs move 3-4KB contiguous segments; weights + const
    blobs go on the scalar HWDGE queue in parallel with x on sync,
    q-slice-0 pieces first (first matmul ~11us in, bounded by the
    framework preamble).
  - KT/QT [c, q] via lhsT = W chunk; bias added in the DVE eviction
    (f32 psum -> bf16).  K_cache leaves bf16 in [c, s] layout; the host
    transposes+casts.  V natural [s, c]; bv folded into the DVE
    eviction as a tensor_tensor add against a host-broadcast row.
    V_aug tiles [k, 80] per head (64 V | 1 ones | 15 zero pad so the AV
    output is XBAR-transposable); ones/pad by engine memset, not DMA (a
    strided sub-word DMA would RMW-race adjacent columns).
  - scores ST[k, q]: per head-pair two row-tiled matmuls (K=64,
    partition offsets 0/64) into one [128, 1024] psum; causal diagonal
    handled by narrowing q and accumulating a -1e6 strict-lower
    triangle into both heads via one strided-output matmul (identity
    lhsT); exp(0.125 x + pad bias) on ACT evicts to bf16 pt.
  - AV: out_unnorm[80, q] += V_aug.T @ PT, lagging the score stream by
    TWO tiles so the PE never blocks on an exp ACT just started.
  - normalization: XBAR DMA transpose [80, 512] -> [q, 4, 80] off the
    PE, reciprocal of the ones column + tensor_scalar_mul on DVE; the
    very last pair instead uses a PE bf16-transpose chain in 256-column
    halves to keep the post-last-exp tail short.
  - schedule: per q-slice qi, attention(qi) is emitted with the qi+1
    projections interleaved as PE filler (in-order queues: fillers must
    never depend on the attention stream or the PE deadlocks); v(3)
    fills the otherwise ACT-bound final window, front-loaded so its
    V_aug tiles land before the avs that consume them.
"""

import numpy as np
import ml_dtypes

BF16 = ml_dtypes.bfloat16

P = 128
S = 2048
HIN = 1024
C = 256  # columns per core = 4 heads * 64
HD = 64
NCORES = 8
HC = HIN // P  # 8 contraction chunks
NKT = S // P  # 16 k-tiles
QW = 512  # q-slice width
NQ = S // QW  # 4 q-slices
NPAIR = C // P  # 2 head-pairs per core
VW = 80  # AV out partitions: 64 V | 1 ones | 15 zero pad (XBAR-aligned)
NEG = -1e6

# const blob layouts
F32C = 276  # [0:2]=bq, [2:4]=bk, [4:20]=padneg, [20:276]=bv bcast
BFC = 640  # [0:256]=ones, [256:512]=tri|tri, [512:640]=eye

_nc_cache = None


def build_nc():
    import concourse.bacc as bacc
    import concourse.mybir as mybir
    from concourse.tile import TileContext
    from contextlib import ExitStack

    f32 = mybir.dt.float32
    f32r = mybir.dt.float32r
    bf16 = mybir.dt.bfloat16
    Exp = mybir.ActivationFunctionType.Exp

    nc = bacc.Bacc(None, target_bir_lowering=False)

    # x and weights arrive pre-packed in SBUF layout ([p, j, ...]) so the
    # load DMAs move 3-4KB contiguous per-partition segments instead of
    # 0.5-1KB ones (the input load is descriptor-rate-bound otherwise)
    xt = nc.declare_dram_parameter("xt", [P, HC * S], bf16, isOutput=False)
    wq = nc.declare_dram_parameter("wq", [P, HC * C], bf16, isOutput=False)
    wk = nc.declare_dram_parameter("wk", [P, HC * C], bf16, isOutput=False)
    wv = nc.declare_dram_parameter("wv", [P, HC * C], bf16, isOutput=False)
    cstf = nc.declare_dram_parameter("cstf", [P, F32C], f32, isOutput=False)
    cstb = nc.declare_dram_parameter("cstb", [P, BFC], bf16, isOutput=False)
    out = nc.declare_dram_parameter("out", [S, C], bf16, isOutput=True)
    kct = nc.declare_dram_parameter("kct", [C, S], bf16, isOutput=True)
    vc = nc.declare_dram_parameter("vc", [S, C], bf16, isOutput=True)

    with TileContext(nc) as tc, ExitStack() as ctx:
        persist = ctx.enter_context(tc.tile_pool(name="persist", bufs=1))
        xt_sb = persist.tile([P, HC, S], bf16)
        wq_sb = persist.tile([P, HC, C], bf16)
        wk_sb = persist.tile([P, HC, C], bf16)
        wv_sb = persist.tile([P, HC, C], bf16)
        cstf_sb = persist.tile([P, F32C], f32)
        cstb_sb = persist.tile([P, BFC], bf16)
        qt_bf = persist.tile([P, NPAIR, S], bf16)
        kt_sb = persist.tile([P, NPAIR, S], bf16)
        va_bf = persist.tile([P, NKT, NPAIR, 2 * VW], bf16)
        ofin = persist.tile([P, NKT, C], bf16)

        bqc_ap = cstf_sb[:, 0:NPAIR]
        bvb_ap = cstf_sb[:, 20 : 20 + C]
        bkc_ap = cstf_sb[:, NPAIR : 2 * NPAIR]
        pn_ap = cstf_sb[:, 4:20]
        ones_ap = cstb_sb[:, 0:C]
        tri2_ap = cstb_sb[:, C : C + 2 * P].rearrange("p (h w) -> p h w", h=2)
        idbf_ap = cstb_sb[:, C + 2 * P : C + 3 * P]

        # weights + consts on the scalar HWDGE queue (ACT is idle this
        # early), x on sync -- both streams in parallel, slice-0 first
        xtv = xt[:].rearrange("p (j q) -> p j q", j=HC)
        wkv = wk[:].rearrange("p (j c) -> p j c", j=HC)
        wqv = wq[:].rearrange("p (j c) -> p j c", j=HC)
        nc.scalar.dma_start(wk_sb[:, 0:4], wkv[:, 0:4])
        nc.scalar.dma_start(wq_sb[:, 0:4], wqv[:, 0:4])
        nc.scalar.dma_start(wk_sb[:, 4:HC], wkv[:, 4:HC])
        nc.scalar.dma_start(wq_sb[:, 4:HC], wqv[:, 4:HC])
        nc.scalar.dma_start(cstf_sb[:], cstf[:])
        nc.scalar.dma_start(cstb_sb[:], cstb[:])
        nc.scalar.dma_start(wv_sb[:], wv[:].rearrange("p (j c) -> p j c", j=HC))
        # q-slice-0 columns per chunk first (fine-grained deps for the
        # first projection), then the whole remainder as 3KB segments
        for j in range(HC):
            nc.sync.dma_start(xt_sb[:, j, 0:QW], xtv[:, j, 0:QW])
        nc.sync.dma_start(xt_sb[:, :, QW:S], xtv[:, :, QW:S])
        # zero the XBAR pad of each V_aug head slice; ones columns
        # (positions 64 and VW+64) by DVE memset -- DMA would RMW-race
        # the adjacent DVE-written V columns
        nc.gpsimd.memset(va_bf[:, :, :, HD + 1 : VW], 0.0)
        nc.gpsimd.memset(va_bf[:, :, :, VW + HD + 1 : 2 * VW], 0.0)
        nc.vector.memset(va_bf[:, :, :, HD : HD + 1], 1.0)
        nc.vector.memset(va_bf[:, :, :, VW + HD : VW + HD + 1], 1.0)

        psum = ctx.enter_context(tc.tile_pool(name="psum", bufs=2, space="PSUM"))
        work = ctx.enter_context(tc.tile_pool(name="work", bufs=3))

        def gen_kt_qt(qi):
            """K+Q projections for q-slice qi; K/Q chunks interleaved so
            adjacent matmuls hit independent psum tiles (PE dual-issue)."""
            qsl = slice(qi * QW, (qi + 1) * QW)
            for p in range(NPAIR):
                csl = slice(p * P, (p + 1) * P)
                ps_k = psum.tile([P, QW], f32, tag="proj", bufs=2, name="k_ps")
                ps_q = psum.tile([P, QW], f32, tag="proj", bufs=2, name="q_ps")
                for j in range(HC):
                    nc.tensor.matmul(
                        ps_k, wk_sb[:, j, csl], xt_sb[:, j, qsl],
                        start=(j == 0), stop=(j == HC - 1),
                    )
                    nc.tensor.matmul(
                        ps_q, wq_sb[:, j, csl], xt_sb[:, j, qsl],
                        start=(j == 0), stop=(j == HC - 1),
                    )
                    yield
                nc.vector.tensor_scalar_add(
                    kt_sb[:, p, qsl], ps_k, bkc_ap[:, p : p + 1]
                )
                nc.vector.tensor_scalar_add(
                    qt_bf[:, p, qsl], ps_q, bqc_ap[:, p : p + 1]
                )
                yield
            # K_cache leaves the chip in kt's [c, s] layout (contiguous
            # DMA); the host transposes+casts it during the gather
            nc.sync.dma_start(
                kct[:, qsl].rearrange("(g p) q -> p g q", p=P),
                kt_sb[:, :, qsl],
            )
            yield

        def gen_v(qi):
            """V projection for k-tiles 4qi..4qi+3, two tiles abreast."""
            vsb = work.tile([P, 4, C], bf16, tag="projsb", bufs=2, name="v_sb")
            for i0 in (4 * qi, 4 * qi + 2):
                pss = []
                for k in range(2):
                    pss.append(
                        psum.tile([P, QW], f32, tag="proj", bufs=2, name="v_ps")[
                            :, :C
                        ]
                    )
                for j in range(HC):
                    for k in range(2):
                        ksl = slice((i0 + k) * P, (i0 + k + 1) * P)
                        nc.tensor.matmul(
                            pss[k], xt_sb[:, j, ksl], wv_sb[:, j, :],
                            start=(j == 0), stop=(j == HC - 1),
                        )
                    yield
                for k in range(2):
                    i = i0 + k
                    sb = vsb[:, i - 4 * qi, :]
                    nc.vector.tensor_add(sb, pss[k], bvb_ap)
                    for p in range(NPAIR):
                        nc.vector.tensor_copy(
                            out=va_bf[:, i, p, 0:HD],
                            in_=sb[:, p * P : p * P + HD],
                        )
                        nc.vector.tensor_copy(
                            out=va_bf[:, i, p, VW : VW + HD],
                            in_=sb[:, p * P + HD : (p + 1) * P],
                        )
                    yield
            nc.sync.dma_start(
                vc[4 * qi * P : (4 * qi + 4) * P, :].rearrange(
                    "(i p) c -> p i c", p=P
                ),
                vsb[:],
            )
            yield

        def gen_attention(qi, is_last=False):
            for p in range(NPAIR):
                av_a = psum.tile([VW, QW], f32, tag="av", bufs=2, name="av_a")
                av_b = psum.tile([VW, QW], f32, tag="av", bufs=2, name="av_b")
                tmax = 4 * qi + 4
                pts = {}

                last = is_last and p == NPAIR - 1

                def emit_av(t):
                    pt, W = pts.pop(t)
                    nc.tensor.matmul(
                        av_a[:, QW - W :], va_bf[:, t, p, 0:VW],
                        pt[:, 0, 0:W], start=(t == 0), stop=(t == tmax - 1),
                    )
                    nc.tensor.matmul(
                        av_b[:, QW - W :], va_bf[:, t, p, VW : 2 * VW],
                        pt[:, 1, 0:W], start=(t == 0), stop=(t == tmax - 1),
                    )

                for t in range(tmax):
                    ksl = slice(t * P, (t + 1) * P)
                    d = t - 4 * qi
                    W = QW if d < 0 else QW - d * P
                    q0 = qi * QW + (0 if d < 0 else d * P)
                    st = psum.tile([P, 2 * QW], f32, tag="st", bufs=2, name="st")
                    nc.tensor.matmul(
                        st[:, 0:W], kt_sb[0:HD, p, ksl],
                        qt_bf[0:HD, p, q0 : q0 + W], start=True, stop=(d < 0),
                    )
                    nc.tensor.matmul(
                        st[:, QW : QW + W], kt_sb[HD:P, p, ksl],
                        qt_bf[HD:P, p, q0 : q0 + W], start=True, stop=(d < 0),
                    )
                    if d >= 0:
                        # causal mask of the leading 128-wide diagonal block
                        # of both heads: one matmul accumulates a -1e6
                        # strict-lower triangle into the two strided regions
                        st4 = st[:].rearrange("p (h w) -> p h w", h=2)[:, :, 0:P]
                        nc.tensor.matmul(
                            st4, idbf_ap, tri2_ap, start=False, stop=True,
                            skip_group_check=True,
                        )
                    pt = work.tile([P, 2, QW], bf16, tag="pt", bufs=4, name="pt")
                    st3 = st[:].rearrange("p (h w) -> p h w", h=2)[:, :, 0:W]
                    nc.scalar.activation(
                        pt[:, :, 0:W], st3, Exp, bias=pn_ap[:, t : t + 1],
                        scale=0.125,
                    )
                    pts[t] = (pt, W)
                    # AV lags the score stream by three tiles so the PE
                    # never blocks on an exp that ACT has just started
                    if t > 2:
                        emit_av(t - 3)
                    yield
                emit_av(tmax - 3)
                emit_av(tmax - 2)
                emit_av(tmax - 1)
                for h, av in ((0, av_a), (1, av_b)):
                    col = p * P + h * HD
                    if last:
                        for hc in range(2):
                            osb = work.tile(
                                [VW, 2 * P], bf16, tag="osb", bufs=2, name="osbh"
                            )
                            nc.vector.tensor_copy(
                                out=osb[:], in_=av[:, hc * 2 * P : (hc + 1) * 2 * P]
                            )
                            for sub2 in range(2):
                                sub = hc * 2 + sub2
                                tr = psum.tile(
                                    [P, P], bf16, tag="av", bufs=2, name="otr"
                                )[:, :VW]
                                nc.tensor.transpose(
                                    tr, osb[:, sub2 * P : (sub2 + 1) * P],
                                    idbf_ap[:VW, :VW],
                                )
                                rcp = work.tile(
                                    [P, 1], f32, tag="rcp", bufs=4, name="rcp"
                                )
                                nc.vector.reciprocal(rcp[:], tr[:, HD : HD + 1])
                                i = 4 * qi + sub
                                nc.vector.tensor_scalar_mul(
                                    ofin[:, i, col : col + HD], tr[:, 0:HD], rcp[:]
                                )
                            yield
                    else:
                        osb = work.tile(
                            [VW, QW], bf16, tag="osb", bufs=2, name="osb"
                        )
                        nc.vector.tensor_copy(out=osb[:], in_=av)
                        yield
                        oT = work.tile([P, 4, VW], bf16, tag="ot", bufs=2, name="ot")
                        nc.sync.dma_start_transpose(oT[:], osb[:])
                        rcp4 = work.tile(
                            [P, 4, 1], f32, tag="rcp4", bufs=2, name="rcp4"
                        )
                        nc.vector.reciprocal(rcp4[:], oT[:, :, HD : HD + 1])
                        yield
                        for sub in range(4):
                            i = 4 * qi + sub
                            nc.vector.tensor_scalar_mul(
                                ofin[:, i, col : col + HD], oT[:, sub, 0:HD],
                                rcp4[:, sub, :],
                            )
                        yield
                # this pair's 128 output columns leave as soon as its
                # normalization is done
                csl = slice(p * P, (p + 1) * P)
                nc.sync.dma_start(
                    out[qi * QW : (qi + 1) * QW, csl].rearrange(
                        "(i p2) c -> p2 i c", p2=P
                    ),
                    ofin[:, 4 * qi : 4 * qi + 4, csl],
                )
                yield

        def interleave(attn, fill, per_step):
            """Emit attention steps with `per_step` filler steps after each.
            Fillers must not depend on the attention stream (the PE queue is
            in-order: a stalled attention op would deadlock fillers emitted
            behind it that it transitively needs)."""
            acc = 0.0
            for _ in attn:
                acc += per_step
                while acc >= 1.0 and fill is not None:
                    try:
                        next(fill)
                        acc -= 1.0
                    except StopIteration:
                        fill = None
                        break
            if fill is not None:
                for _ in fill:
                    pass

        def chain(*gens):
            for g in gens:
                yield from g

        # warmup: slice-0 projections (nothing to overlap them with)
        for _ in chain(gen_kt_qt(0), gen_v(0)):
            pass
        # steady state: attention(qi) overlapped with upcoming projections
        # as PE filler.  v(3) runs inside window 3 (front-loaded: its V_aug
        # tiles are consumed from step 13 on, and window 3 has no other
        # filler) so the ACT-bound last window keeps the PE busy.
        nsteps = lambda qi: NPAIR * (4 * qi + 4 + 7)
        interleave(gen_attention(0), chain(gen_kt_qt(1), gen_v(1)), 42 / nsteps(0))
        interleave(gen_attention(1), chain(gen_kt_qt(2), gen_v(2)), 42 / nsteps(1))
        interleave(gen_attention(2), gen_kt_qt(3), 19 / nsteps(2))
        interleave(gen_attention(3, is_last=True), gen_v(3), 3.0)

    nc.finalize()
    return nc


def get_nc():
    global _nc_cache
    if _nc_cache is None:
        _nc_cache = build_nc()
    return _nc_cache


def make_in_maps(x, pad_mask, Wq, bq, Wk, bk, Wv, bv):
    x = np.asarray(x, np.float32)
    pad_mask = np.asarray(pad_mask, np.float32)
    Wq = np.asarray(Wq, BF16)
    bq = np.asarray(bq, np.float32)
    Wk = np.asarray(Wk, BF16)
    bk = np.asarray(bk, np.float32)
    Wv = np.asarray(Wv, BF16)
    bv = np.asarray(bv, np.float32)
    tri = (np.arange(P)[None, :] < np.arange(P)[:, None]).astype(np.float32) * NEG
    in_maps = []
    def pack(a):
        # [HIN, n] -> SBUF layout [P, HC * n]
        n = a.shape[1]
        return np.ascontiguousarray(
            a.reshape(HC, P, n).transpose(1, 0, 2).reshape(P, HC * n)
        )

    for c in range(NCORES):
        b, g = divmod(c, 4)
        cols = slice(g * C, (g + 1) * C)
        xt = pack(np.ascontiguousarray(x[b].T)).astype(BF16)  # [P, HC*S]
        pn = ((pad_mask[b] - 1.0) * 1e6).reshape(NKT, P).T  # [P, NKT]
        cstf = np.empty((P, F32C), np.float32)
        cstf[:, 0:NPAIR] = bq[cols].reshape(NPAIR, P).T
        cstf[:, NPAIR : 2 * NPAIR] = bk[cols].reshape(NPAIR, P).T
        cstf[:, 4:20] = pn
        cstf[:, 20 : 20 + C] = bv[cols].astype(np.float32)[None, :]
        cstb = np.zeros((P, BFC), np.float32)
        cstb[:, 0:C] = 1.0
        cstb[:, C : C + P] = tri
        cstb[:, C + P : C + 2 * P] = tri
        cstb[:, C + 2 * P : C + 3 * P] = np.eye(P, dtype=np.float32)
        in_maps.append(
            dict(
                xt=xt,
                wq=pack(Wq[:, cols]),
                wk=pack(Wk[:, cols]),
                wv=pack(Wv[:, cols]),
                cstf=cstf,
                cstb=cstb.astype(BF16),
            )
        )
    return in_maps


def gather(results):
    B = 2
    out = np.empty((B, S, HIN), np.float32)
    kcache = np.empty((B, S, HIN), np.float32)
    vcache = np.empty((B, S, HIN), np.float32)
    for c in range(NCORES):
        b, g = divmod(c, 4)
        cols = slice(g * C, (g + 1) * C)
        out[b, :, cols] = results[c]["out"].astype(np.float32)
        kcache[b, :, cols] = results[c]["kct"].astype(np.float32).T
        vcache[b, :, cols] = results[c]["vc"].astype(np.float32)
    return out, kcache, vcache


def kernel(x, pad_mask, Wq, bq, Wk, bk, Wv, bv):
    from concourse.bass_utils import run_bass_kernel_spmd

    nc = get_nc()
    in_maps = make_in_maps(x, pad_mask, Wq, bq, Wk, bk, Wv, bv)
    res = run_bass_kernel_spmd(nc, in_maps, list(range(NCORES)))
    return gather(res.results)
